# revision 2
# baseline (speedup 1.0000x reference)
"""Raw-bass Trainium2 kernel for nn_CriticTab, v5.

v4 + latency micro-optimizations:
- obs load split across Sync+Scalar HWDGE rings (parallel descriptor gen,
  earlier data landing).
- dummy SWDGE gather during the obs-DMA window (pays the first-call
  INDIRECT1D setup overhead off the critical path).
- idx computed in two column stages so gathers 0/1 dispatch before the
  shift/reduce for groups 2/3 has finished.
- output store split across Sync+Scalar so issue + completion overlap.
"""

import numpy as np

B, D, N = 4096, 16, 65536
N_CORES = 8
BS = B // N_CORES
P = 128
G = BS // P
H = P // 2  # partition half

_CACHE = {}

LAST_RESULT = None


def build_program():
    if "nc" in _CACHE:
        return _CACHE["nc"]

    import concourse.bacc as bacc
    import concourse.bass as bass
    import concourse.mybir as mybir

    nc = bacc.Bacc("TRN2", debug=False, enable_asserts=False, num_devices=N_CORES)
    obs_d = nc.dram_tensor("obs", [BS, D], mybir.dt.int32, kind="ExternalInput")
    v_d = nc.dram_tensor("v", [N, 1], mybir.dt.float32, kind="ExternalInput")
    out_d = nc.dram_tensor("out", [BS], mybir.dt.float32, kind="ExternalOutput")

    obs_r = obs_d[:].rearrange("(p g) d -> p (g d)", p=P)
    out_r = out_d[:].rearrange("(p g) -> p g", p=P)

    with (
        nc.semaphore("s_obs") as s_obs,
        nc.semaphore("s_iota") as s_iota,
        nc.semaphore("s_tmp") as s_tmp,
        nc.semaphore("s_idx") as s_idx,
        nc.semaphore("s_dum") as s_dum,
        nc.semaphore("s_dumg") as s_dumg,
        nc.semaphore("s_g") as s_g,
        nc.semaphore("s_done") as s_done,
        nc.semaphore("s_done2") as s_done2,
        nc.sbuf_tensor("obs_t", [P, G * D], mybir.dt.int32) as obs_t,
        nc.sbuf_tensor("sh_t", [P, G * D], mybir.dt.int32) as sh_t,
        nc.sbuf_tensor("prod_t", [P, G * D], mybir.dt.int32) as prod_t,
        nc.sbuf_tensor("idx_t", [P, G], mybir.dt.int32) as idx_t,
        nc.sbuf_tensor("zoff_t", [P, 1], mybir.dt.int32) as zoff_t,
        nc.sbuf_tensor("g_t", [P, G], mybir.dt.float32) as g_t,
        nc.sbuf_tensor("gdum_t", [P, 1], mybir.dt.float32) as gdum_t,
    ):
        # obs load: two partition halves on two HWDGE rings.
        nc.sync.dma_start(out=obs_t[0:H, :], in_=obs_r[0:H, :]).then_inc(s_obs, 16)
        nc.scalar.dma_start(out=obs_t[H:P, :], in_=obs_r[H:P, :]).then_inc(s_obs, 16)

        # GpSimd (during the obs DMA): shift table, zero offsets, and a
        # dummy gather that absorbs the SWDGE first-call overhead.
        nc.gpsimd.iota(
            sh_t[:], pattern=[[0, G], [1, D]], channel_multiplier=0
        ).then_inc(s_iota, 1)
        nc.gpsimd.memset(zoff_t[:], 0).then_inc(s_dum, 1)
        nc.gpsimd.wait_ge(s_dum, 1)
        nc.gpsimd.indirect_dma_start(
            out=gdum_t[:, 0:1],
            out_offset=None,
            in_=v_d[:],
            in_offset=bass.IndirectOffsetOnAxis(ap=zoff_t[:, 0:1], axis=0),
            oob_is_err=False,
        ).then_inc(s_dumg, 16)

        # Vector: idx = sum_d obs<<d, staged: groups 0-1 first.
        nc.vector.wait_ge(s_iota, 1)
        nc.vector.wait_ge(s_obs, 32)
        with nc.allow_low_precision(reason="exact small-int bit packing"):
            h = G * D // 2
            nc.vector.tensor_tensor(
                out=prod_t[:, 0:h],
                in0=obs_t[:, 0:h],
                in1=sh_t[:, 0:h],
                op=mybir.AluOpType.logical_shift_left,
            ).then_inc(s_tmp, 1)
            nc.vector.tensor_tensor(
                out=prod_t[:, h:],
                in0=obs_t[:, h:],
                in1=sh_t[:, h:],
                op=mybir.AluOpType.logical_shift_left,
            ).then_inc(s_tmp, 1)
            nc.vector.wait_ge(s_tmp, 1)
            nc.vector.tensor_reduce(
                out=idx_t[:, 0 : G // 2],
                in_=prod_t[:, 0:h].rearrange("p (g d) -> p g d", d=D),
                axis=mybir.AxisListType.X,
                op=mybir.AluOpType.add,
            ).then_inc(s_idx, 1)
            nc.vector.wait_ge(s_tmp, 2)
            nc.vector.tensor_reduce(
                out=idx_t[:, G // 2 : G],
                in_=prod_t[:, h:].rearrange("p (g d) -> p g d", d=D),
                axis=mybir.AxisListType.X,
                op=mybir.AluOpType.add,
            ).then_inc(s_idx, 1)

        # GpSimd: four gathers; 0/1 go as soon as the first reduce lands.
        nc.gpsimd.wait_ge(s_idx, 1)
        for j in range(G // 2):
            nc.gpsimd.indirect_dma_start(
                out=g_t[:, j : j + 1],
                out_offset=None,
                in_=v_d[:],
                in_offset=bass.IndirectOffsetOnAxis(ap=idx_t[:, j : j + 1], axis=0),
                oob_is_err=False,
            ).then_inc(s_g, 16)
        nc.gpsimd.wait_ge(s_idx, 2)
        for j in range(G // 2, G):
            nc.gpsimd.indirect_dma_start(
                out=g_t[:, j : j + 1],
                out_offset=None,
                in_=v_d[:],
                in_offset=bass.IndirectOffsetOnAxis(ap=idx_t[:, j : j + 1], axis=0),
                oob_is_err=False,
            ).then_inc(s_g, 16)

        # Store: two partition halves on two HWDGE rings.
        nc.sync.wait_ge(s_g, 64)
        nc.sync.dma_start(out=out_r[0:H, :], in_=g_t[0:H, :]).then_inc(s_done, 16)
        nc.scalar.wait_ge(s_g, 64)
        nc.scalar.dma_start(out=out_r[H:P, :], in_=g_t[H:P, :]).then_inc(s_done2, 16)
        nc.sync.wait_ge(s_done, 16)
        nc.scalar.wait_ge(s_done2, 16)

    nc.compile()
    _CACHE["nc"] = nc
    return nc


def _fold_table(mask: np.ndarray, v: np.ndarray) -> np.ndarray:
    pw = 1 << np.arange(D, dtype=np.int64)
    m01 = (np.asarray(mask).astype(np.int64) + 1) // 2
    keys = (m01 * pw[None, :]).sum(axis=1)
    if np.array_equal(keys, np.arange(N, dtype=np.int64)):
        return v
    table = np.zeros(N, dtype=np.float32)
    np.add.at(table, keys, v)
    return table


def kernel(obs, mask, v):
    global LAST_RESULT
    from concourse.bass_utils import run_bass_kernel_spmd

    obs = np.ascontiguousarray(np.asarray(obs), dtype=np.int32)
    v = np.ascontiguousarray(np.asarray(v), dtype=np.float32)
    table = np.ascontiguousarray(_fold_table(mask, v)).reshape(N, 1)

    nc = build_program()
    in_maps = [
        {"obs": obs[i * BS : (i + 1) * BS], "v": table} for i in range(N_CORES)
    ]
    res = run_bass_kernel_spmd(nc, in_maps, list(range(N_CORES)))
    LAST_RESULT = res
    return np.concatenate(
        [res.results[i]["out"].reshape(BS) for i in range(N_CORES)]
    )
